# revision 19
# baseline (speedup 1.0000x reference)
"""Trainium2 Bass kernel for nn_CrossAtt (dual cross-attention + 3x3 conv + BN + ReLU).

Sharding: 8 cores = (sample s in 0..3) x (h-half in 0..1). Each core computes
its 32 output rows plus a 1-row attention halo on each side (34 rows = 2176
query positions, host-zero-padded so the program is SPMD-uniform), then runs
the 3x3 conv locally. No collectives.

Device layout choices:
- scoresT [m, n] comes straight off the PE (lhsT=k zero-padded to K=128,
  rhs=q), so softmax needs no transpose of the 4096x2176 matrix.
- exp on ScalarE (no max subtraction; |scores| <~ 5 so fp32 exp is safe).
- AV: out^T[n, 257] = expT.T @ [vT | ones]; col 256 accumulates the softmax
  denominator S for free.
- normalize by (gamma * mask / S) as a per-partition scalar; mask zeroes the
  fake padded query rows. PE-transposes the small [n,256] result to [256,n].
- residual + gamma*bv bias are folded into the host-prepared x?r inputs.
- conv3x3 = 9 shifted matmuls over a [512, 35*66] zero-padded cat buffer;
  BN+ReLU fused into one activation (scale=inv, bias=beta per partition).
"""
import sys

if "/opt/trn_rl_repo" not in sys.path:
    sys.path.insert(0, "/opt/trn_rl_repo")

import numpy as np

import concourse.bass as bass
import concourse.bacc as bacc
import concourse.mybir as mybir
import concourse.tile as tile
from concourse.bass import ds, ts
from concourse.bass_utils import run_bass_kernel_spmd

F32 = mybir.dt.float32
F32R = mybir.dt.float32r  # same bits as fp32; 1 cycle/row PE mode (vs 4 for fp32)
EPS = 1e-5
P = 128
C = 256          # channels
M = 4096         # key/value positions (64*64)
NQ = 2176        # query positions per core (34 rows * 64), host padded
NROWS = 35       # cat_pad rows (34 data + 1 zero)
WPAD = 66        # cat_pad row width (64 + 2 zero cols)
# all moving dims >= 256 so the fp32r fast path applies
ATT_BLOCKS = [(i * 256, 256) for i in range(8)] + [(2048, 128)]
QWINS = [(0, 512), (512, 512), (1024, 512), (1536, 384), (1920, 256)]
CONV_WINS = [(1, 512), (513, 512), (1025, 512), (1537, 318), (1855, 256)]

_CACHE = {}


def _wins(total, w):
    return [(i, min(w, total - i)) for i in range(0, total, w)]


def _mm(nc, out, lhsT, rhs, **kw):
    nc.tensor.matmul(out, lhsT, rhs, **kw)


def _declare_io(nc):
    t = {}
    inp = lambda name, shape, dt=F32: t.__setitem__(
        name, nc.dram_tensor(name, shape, dt, kind="ExternalInput"))
    out = lambda name, shape, dt=F32: t.__setitem__(
        name, nc.dram_tensor(name, shape, dt, kind="ExternalOutput"))
    # fp32r = same 32-bit data; matmul operands must be declared fp32r end-to-end
    inp("x1", [C, M], F32R); inp("x2", [C, M], F32R)
    inp("x1q", [C, NQ], F32R); inp("x2q", [C, NQ], F32R)
    inp("x1r", [C, NQ]); inp("x2r", [C, NQ])
    inp("maskg", [P, 17])
    inp("wq1T", [P, 2, 32], F32R); inp("wq2T", [P, 2, 32], F32R)
    inp("wk1T", [P, 2, 64], F32R); inp("wk2T", [P, 2, 64], F32R)
    inp("wv1T", [P, 2, C], F32R); inp("wv2T", [P, 2, C], F32R)
    inp("bq1", [32, 1]); inp("bq2", [32, 1])
    inp("bk1", [64, 1]); inp("bk2", [64, 1])
    inp("cinv", [P, 2]); inp("cbeta", [P, 2])
    inp("wct", [3, 3, 2 * C, C], F32R)
    inp("ident", [P, P])
    out("feat", [C, 32, 64]); out("o1", [C, 32, 64], F32R); out("o2", [C, 32, 64], F32R)
    return t


def _emit(nc, tc, t, ctx):
    big = ctx.enter_context(tc.tile_pool(name="big", bufs=3))
    kqp = ctx.enter_context(tc.tile_pool(name="kq", bufs=1))
    sing = ctx.enter_context(tc.tile_pool(name="sing", bufs=1))
    expp = ctx.enter_context(tc.tile_pool(name="expp", bufs=3))
    normp = ctx.enter_context(tc.tile_pool(name="normp", bufs=3))
    scalp = ctx.enter_context(tc.tile_pool(name="scalp", bufs=4))
    resp = ctx.enter_context(tc.tile_pool(name="resp", bufs=4))
    wcp = ctx.enter_context(tc.tile_pool(name="wcp", bufs=12))
    psA = ctx.enter_context(tc.tile_pool(name="psA", bufs=2, space="PSUM"))
    psS = ctx.enter_context(tc.tile_pool(name="psS", bufs=2, space="PSUM"))

    BIG_SHAPE_BYTES = [P, 4 * NROWS * WPAD]  # cat_pad is the largest big tile

    # ---- constants / weights to SBUF ----
    idt = sing.tile([P, P], F32)
    nc.sync.dma_start(out=idt, in_=t["ident"][:])
    wq_sb, wk_sb, wv_sb, bq_sb, bk_sb = {}, {}, {}, {}, {}
    for b in (1, 2):
        wq_sb[b] = sing.tile([P, 2, 32], F32R, tag=f"wq{b}", name=f"wq{b}")
        nc.sync.dma_start(out=wq_sb[b], in_=t[f"wq{b}T"][:])
        wk_sb[b] = sing.tile([P, 2, 64], F32R, tag=f"wk{b}", name=f"wk{b}")
        nc.sync.dma_start(out=wk_sb[b], in_=t[f"wk{b}T"][:])
        wv_sb[b] = sing.tile([P, 2, C], F32R, tag=f"wv{b}", name=f"wv{b}")
        nc.sync.dma_start(out=wv_sb[b], in_=t[f"wv{b}T"][:])
        bq_sb[b] = sing.tile([32, 1], F32, tag=f"bq{b}", name=f"bq{b}")
        nc.sync.dma_start(out=bq_sb[b], in_=t[f"bq{b}"][:])
        bk_sb[b] = sing.tile([64, 1], F32, tag=f"bk{b}", name=f"bk{b}")
        nc.sync.dma_start(out=bk_sb[b], in_=t[f"bk{b}"][:])
    cinv_sb = sing.tile([P, 2], F32, tag="cinv")
    nc.sync.dma_start(out=cinv_sb, in_=t["cinv"][:])
    cbeta_sb = sing.tile([P, 2], F32, tag="cbeta")
    nc.sync.dma_start(out=cbeta_sb, in_=t["cbeta"][:])
    maskg_sb = sing.tile([P, 17], F32, tag="maskg")
    nc.sync.dma_start(out=maskg_sb, in_=t["maskg"][:])

    # ---- load x1, x2 (two column-half DMAs so the PE can start earlier) ----
    def load_x(name):
        x_sb = big.tile(BIG_SHAPE_BYTES, F32R, tag="big")
        xv = x_sb[:, : 2 * M].rearrange("p (kc n) -> p kc n", kc=2)
        src_ap = t[name][:].rearrange("(kc p) n -> p kc n", p=P)
        for c0 in range(0, M, 1024):
            nc.sync.dma_start(out=xv[:, :, ds(c0, 1024)],
                              in_=src_ap[:, :, ds(c0, 1024)])
        return xv

    x1_sb = load_x("x1")
    x2_sb = load_x("x2")

    # ---- k projections: k_b = wk_b @ x_b + bk_b, stored [128(c pad0), 4096] ----
    k_sb = {}
    for b, x_sb in ((1, x1_sb), (2, x2_sb)):
        kp = kqp.tile([P, M], F32R, tag=f"k{b}")
        for w0, ww in _wins(M, 512):
            ps = psS.tile([P, 1024], F32, tag="sc")
            for kc in range(2):
                _mm(nc, ps[0:64, :ww], wk_sb[b][:, kc, :],
                    x_sb[:, kc, ds(w0, ww)],
                    start=(kc == 0), stop=(kc == 1))
            nc.vector.tensor_scalar_add(kp[0:64, ds(w0, ww)], ps[0:64, :ww], bk_sb[b])
        k_sb[b] = kp

    # ---- vT projections: vT_b[m, c] = x_b.T @ wv_bT (no bias), plus ones col ----
    def make_vt(x_sb, b):
        vt = big.tile(BIG_SHAPE_BYTES, F32R, tag="big")
        vtv = vt[:, : 32 * 258].rearrange("p (mi c) -> p mi c", mi=32)
        nc.vector.memset(vtv[:, :, 256:257].bitcast(F32), 1.0)
        nc.vector.memset(vtv[:, :, 257:258].bitcast(F32), 0.0)
        for mi in range(32):
            ps_full = psS.tile([P, 1024], F32, tag="sc", name="vtps")
            ps = ps_full[:, :256]
            for kc in range(2):
                _mm(nc, ps, x_sb[:, kc, ts(mi, P)], wv_sb[b][:, kc, :],
                    start=(kc == 0), stop=(kc == 1))
            nc.vector.tensor_copy(out=vtv[:, mi, 0:256], in_=ps)
        return vtv

    # ---- q projection (shared by both branches): qp [128(c pad0), 2176] ----
    qp = kqp.tile([P, NQ], F32R, tag="qp")

    def q_half(name, b, row0):
        xq = big.tile(BIG_SHAPE_BYTES, F32R, tag="big")
        xqv = xq[:, : 2 * NQ].rearrange("p (kc n) -> p kc n", kc=2)
        xq_src = t[name][:].rearrange("(kc p) n -> p kc n", p=P)
        nc.sync.dma_start(out=xqv[:, :, 0:1088], in_=xq_src[:, :, 0:1088])
        nc.sync.dma_start(out=xqv[:, :, 1088:NQ], in_=xq_src[:, :, 1088:NQ])
        for w0, ww in QWINS:
            ps = psS.tile([P, 1024], F32, tag="sc")
            for kc in range(2):
                _mm(nc, ps[0:32, :ww], wq_sb[b][:, kc, :],
                    xqv[:, kc, ds(w0, ww)],
                    start=(kc == 0), stop=(kc == 1))
            nc.vector.tensor_scalar_add(qp[row0:row0 + 32, ds(w0, ww)],
                                        ps[0:32, :ww], bq_sb[b])

    q_half("x1q", 1, 0)
    vt1 = make_vt(x1_sb, 1)
    q_half("x2q", 2, 32)
    vt2 = make_vt(x2_sb, 2)

    # ---- cat_pad buffer [128, 4, 35*66], zeroed ----
    cat = big.tile(BIG_SHAPE_BYTES, F32R, tag="big")
    catv = cat[:].rearrange("p (i f) -> p i f", i=4)
    cat_r = cat[:].rearrange("p (i r w) -> p i r w", i=4, w=WPAD)
    nc.gpsimd.memset(cat[:].bitcast(F32), 0.0)

    # ---- attention branches ----
    for b, (kp, vtv, xr_name) in enumerate(
            [(k_sb[1], vt1, "x1r"), (k_sb[2], vt2, "x2r")]):
        for n0, nw in ATT_BLOCKS:
            nsub = nw // P
            g = 1024 // nw  # m-iters per exp group (4 for nw=256, 8 for 128)
            av = psA.tile([P, 1024], F32, tag="av")

            def flush_av(pend, av=av, vtv=vtv, nw=nw, nsub=nsub):
                g0, ex = pend
                for u in range(1024 // nw):
                    pmi = g0 + u
                    for j in range(nsub):
                        _mm(nc, av[:, ds(j * 512, 258)],
                            ex[:, ds(u * nw + j * P, P)], vtv[:, pmi, :],
                            start=(pmi == 0), stop=(pmi == 31))

            pend = None
            for g0 in range(0, 32, g):
                sc = psS.tile([P, 1024], F32, tag="sc")
                for u in range(g):
                    mi = g0 + u
                    _mm(nc, sc[:, ds(u * nw, nw)],
                        kp[0:64, ts(mi, P)], qp[0:64, ds(n0, nw)],
                        start=True, stop=True)
                ex = expp.tile([P, 1024], F32R, tag="ex")
                nc.scalar.activation(ex, sc, mybir.ActivationFunctionType.Exp)
                if pend is not None:
                    flush_av(pend)
                pend = (g0, ex)
            flush_av(pend)

            # epilogue per n-chunk of 128; transposes reuse the consumed AV bank
            for j in range(nsub):
                nch = n0 // P + j
                rs = scalp.tile([P, 1], F32, tag="rs")
                nc.vector.reciprocal(rs, av[:, ds(j * 512 + 256, 1)])
                nc.vector.tensor_mul(out=rs, in0=rs,
                                     in1=maskg_sb[:, ds(nch, 1)])
                nt = normp.tile([P, 256], F32, tag="nt")
                nc.vector.tensor_scalar_mul(nt, av[:, ds(j * 512, 256)], rs)
                rt = resp.tile([P, 2, P], F32, tag="rt")
                nc.sync.dma_start(
                    out=rt,
                    in_=t[xr_name][:].rearrange("(cc p) n -> p cc n", p=P)
                    [:, :, ts(nch, P)])
                for cc in range(2):
                    tp = av[:, ds(j * 512 + cc * P, P)]
                    nc.tensor.transpose(tp, nt[:, ts(cc, P)], idt)
                    nc.vector.tensor_add(
                        out=cat_r[:, 2 * b + cc, ds(2 * nch, 2), ds(1, 64)],
                        in0=tp.rearrange("p (r w) -> p r w", w=64),
                        in1=rt[:, cc, :].rearrange("p (r w) -> p r w", w=64))

        # write out this branch's attention output (rows 1..33 = the 32 real rows)
        ov = t[f"o{b + 1}"][:].rearrange("(cc p) h w -> p cc h w", p=P)
        for cc in range(2):
            nc.sync.dma_start(out=ov[:, cc],
                              in_=cat_r[:, 2 * b + cc, ds(1, 32), ds(1, 64)])

    # ---- conv 3x3 + BN + ReLU ----
    feat = big.tile(BIG_SHAPE_BYTES, F32, tag="big")
    featv = feat[:, : 2 * 2112].rearrange("p (o f) -> p o f", o=2)
    feat_r = feat[:, : 2 * 2112].rearrange("p (o r w) -> p o r w", o=2, w=WPAD)
    for oc in range(2):
        avc1 = psA.tile([P, 1024], F32, tag="av")
        avc2 = psA.tile([P, 1024], F32, tag="av")
        last = psS.tile([P, 1024], F32, tag="sc")

        def conv_dst(wi, ww, avc1=avc1, avc2=avc2, last=last):
            if wi < 2:
                return avc1[:, ds(wi * 512, ww)]
            if wi < 4:
                return avc2[:, ds((wi - 2) * 512, ww)]
            return last[:, :ww]

        wts = {}
        for ic in range(4):
            for tap in range(9):
                wt = wcp.tile([P, P], F32R, tag="wt", name=f"wt{oc}_{ic}_{tap}")
                nc.sync.dma_start(
                    out=wt, in_=t["wct"][tap // 3, tap % 3,
                                         ts(ic, P), ts(oc, P)])
                wts[(ic, tap)] = wt
        for ic in range(4):
            for tap in range(9):
                off = (tap // 3) * WPAD + (tap % 3) - 1
                for wi, (ws, ww) in enumerate(CONV_WINS):
                    _mm(nc, conv_dst(wi, ww), wts[(ic, tap)],
                        catv[:, ic, ds(ws + off, ww)],
                        start=(ic == 0 and tap == 0),
                        stop=(ic == 3 and tap == 8))
        for wi, (ws, ww) in enumerate(CONV_WINS):
            nc.scalar.activation(featv[:, oc, ds(ws, ww)], conv_dst(wi, ww),
                                 mybir.ActivationFunctionType.Relu,
                                 bias=cbeta_sb[:, ds(oc, 1)],
                                 scale=cinv_sb[:, ds(oc, 1)])
    fv = t["feat"][:].rearrange("(cc p) h w -> p cc h w", p=P)
    for oc in range(2):
        nc.sync.dma_start(out=fv[:, oc], in_=feat_r[:, oc, :, ds(1, 64)])


def _build():
    if "nc" in _CACHE:
        return _CACHE["nc"]
    nc = bacc.Bacc(None, target_bir_lowering=False)
    t = _declare_io(nc)
    from contextlib import ExitStack
    with tile.TileContext(nc) as tc, ExitStack() as ctx:
        _emit(nc, tc, t, ctx)
    nc.finalize()
    _CACHE["nc"] = nc
    return nc


def _prep_host(inputs):
    d = {k: np.ascontiguousarray(np.asarray(v, np.float32)) for k, v in inputs.items()}
    gamma = float(d["gamma"].reshape(-1)[0])
    inv = d["bn_scale"] / np.sqrt(d["bn_var"] + EPS)
    beta = d["bn_bias"] - d["bn_mean"] * inv

    def chunked(w):  # [256, o] -> [128, 2, o]
        return np.ascontiguousarray(w.reshape(2, P, -1).transpose(1, 0, 2))

    shared = {
        "wq1T": chunked(d["wq1"].T), "wq2T": chunked(d["wq2"].T),
        "wk1T": chunked(d["wk1"].T), "wk2T": chunked(d["wk2"].T),
        "wv1T": chunked(d["wv1"].T), "wv2T": chunked(d["wv2"].T),
        "bq1": d["bq1"].reshape(32, 1).copy(), "bq2": d["bq2"].reshape(32, 1).copy(),
        "bk1": d["bk1"].reshape(64, 1).copy(), "bk2": d["bk2"].reshape(64, 1).copy(),
        "cinv": np.ascontiguousarray(inv.reshape(2, P).T),
        "cbeta": np.ascontiguousarray(beta.reshape(2, P).T),
        "wct": np.ascontiguousarray(d["w_cat"].transpose(2, 3, 1, 0)),
        "ident": np.eye(P, dtype=np.float32),
    }
    gbv = {1: gamma * d["bv1"], 2: gamma * d["bv2"]}

    in_maps = []
    for core in range(8):
        s, half = core // 2, core % 2
        h0 = 32 * half
        x1 = np.ascontiguousarray(d["input1"][s].reshape(C, M))
        x2 = np.ascontiguousarray(d["input2"][s].reshape(C, M))
        n_lo, n_hi = (h0 - 1) * 64, (h0 + 33) * 64
        lo_pad, hi_pad = max(0, -n_lo), max(0, n_hi - M)
        sl = slice(n_lo + lo_pad, n_hi - hi_pad)

        def pad_slice(x, add=None):
            o = np.zeros((C, NQ), np.float32)
            body = x[:, sl]
            if add is not None:
                body = body + add[:, None]
            o[:, lo_pad:NQ - hi_pad] = body
            return o

        maskg = np.zeros(NQ, np.float32)
        maskg[lo_pad:NQ - hi_pad] = gamma
        m = dict(shared)
        m.update({
            "x1": x1, "x2": x2,
            "x1q": pad_slice(x1), "x2q": pad_slice(x2),
            "x1r": pad_slice(x1, gbv[1]), "x2r": pad_slice(x2, gbv[2]),
            "maskg": np.ascontiguousarray(maskg.reshape(17, P).T),
        })
        in_maps.append(m)
    return in_maps


def kernel(**inputs):
    nc = _build()
    in_maps = _prep_host(inputs)
    res = run_bass_kernel_spmd(nc, in_maps, core_ids=list(range(8)))
    _CACHE["last_results"] = res
    feat = np.zeros((4, C, 64, 64), np.float32)
    o1 = np.zeros((4, C, 64, 64), np.float32)
    o2 = np.zeros((4, C, 64, 64), np.float32)
    for core in range(8):
        s, half = core // 2, core % 2
        r = res.results[core]
        feat[s, :, 32 * half:32 * half + 32] = r["feat"]
        o1[s, :, 32 * half:32 * half + 32] = r["o1"]
        o2[s, :, 32 * half:32 * half + 32] = r["o2"]
    return (feat, o1, o2)


# revision 21
# speedup vs baseline: 38918.8025x; 38918.8025x over previous
"""Trainium2 Bass kernel for nn_CrossAtt (dual cross-attention + 3x3 conv + BN + ReLU).

Sharding: 8 cores = (sample s in 0..3) x (h-half in 0..1). Each core computes
its 32 output rows plus a 1-row attention halo on each side (34 rows = 2176
query positions, host-zero-padded so the program is SPMD-uniform), then runs
the 3x3 conv locally. No collectives.

Device layout choices:
- scoresT [m, n] comes straight off the PE (lhsT=k zero-padded to K=128,
  rhs=q), so softmax needs no transpose of the 4096x2176 matrix.
- exp on ScalarE (no max subtraction; |scores| <~ 5 so fp32 exp is safe).
- AV: out^T[n, 257] = expT.T @ [vT | ones]; col 256 accumulates the softmax
  denominator S for free.
- normalize by (gamma * mask / S) as a per-partition scalar; mask zeroes the
  fake padded query rows. PE-transposes the small [n,256] result to [256,n].
- residual + gamma*bv bias are folded into the host-prepared x?r inputs.
- conv3x3 = 9 shifted matmuls over a [512, 35*66] zero-padded cat buffer;
  BN+ReLU fused into one activation (scale=inv, bias=beta per partition).
"""
import sys

if "/opt/trn_rl_repo" not in sys.path:
    sys.path.insert(0, "/opt/trn_rl_repo")

import numpy as np

import concourse.bass as bass
import concourse.bacc as bacc
import concourse.mybir as mybir
import concourse.tile as tile
from concourse.bass import ds, ts
from concourse.bass_utils import run_bass_kernel_spmd

F32 = mybir.dt.float32
F32R = mybir.dt.float32r  # same bits as fp32; 1 cycle/row PE mode (vs 4 for fp32)
EPS = 1e-5
P = 128
C = 256          # channels
M = 4096         # key/value positions (64*64)
NQ = 2176        # query positions per core (34 rows * 64), host padded
NROWS = 35       # cat_pad rows (34 data + 1 zero)
WPAD = 66        # cat_pad row width (64 + 2 zero cols)
# all moving dims >= 256 so the fp32r fast path applies
ATT_BLOCKS = [(i * 256, 256) for i in range(8)] + [(2048, 128)]
QWINS = [(0, 512), (512, 512), (1024, 512), (1536, 384), (1920, 256)]
CONV_WINS = [(1, 512), (513, 512), (1025, 512), (1537, 318), (1855, 256)]

_CACHE = {}


def _wins(total, w):
    return [(i, min(w, total - i)) for i in range(0, total, w)]


def _mm(nc, out, lhsT, rhs, **kw):
    nc.tensor.matmul(out, lhsT, rhs, **kw)


def _declare_io(nc):
    t = {}
    inp = lambda name, shape, dt=F32: t.__setitem__(
        name, nc.dram_tensor(name, shape, dt, kind="ExternalInput"))
    out = lambda name, shape, dt=F32: t.__setitem__(
        name, nc.dram_tensor(name, shape, dt, kind="ExternalOutput"))
    # fp32r = same 32-bit data; matmul operands must be declared fp32r end-to-end
    inp("x1", [C, M], F32R); inp("x2", [C, M], F32R)
    inp("x1q", [C, NQ], F32R); inp("x2q", [C, NQ], F32R)
    inp("x1r", [C, NQ]); inp("x2r", [C, NQ])
    inp("maskg", [P, 17])
    inp("wq1T", [P, 2, 32], F32R); inp("wq2T", [P, 2, 32], F32R)
    inp("wk1T", [P, 2, 64], F32R); inp("wk2T", [P, 2, 64], F32R)
    inp("wv1T", [P, 2, C], F32R); inp("wv2T", [P, 2, C], F32R)
    inp("bq1", [32, 1]); inp("bq2", [32, 1])
    inp("bk1", [64, 1]); inp("bk2", [64, 1])
    inp("cinv", [P, 2]); inp("cbeta", [P, 2])
    inp("wct", [3, 3, 2 * C, C], F32R)
    inp("ident", [P, P])
    out("feat", [C, 32, 64]); out("o1", [C, 32, 64], F32R); out("o2", [C, 32, 64], F32R)
    return t


def _emit(nc, tc, t, ctx):
    big = ctx.enter_context(tc.tile_pool(name="big", bufs=3))
    kqp = ctx.enter_context(tc.tile_pool(name="kq", bufs=1))
    sing = ctx.enter_context(tc.tile_pool(name="sing", bufs=1))
    expp = ctx.enter_context(tc.tile_pool(name="expp", bufs=3))
    normp = ctx.enter_context(tc.tile_pool(name="normp", bufs=3))
    scalp = ctx.enter_context(tc.tile_pool(name="scalp", bufs=4))
    resp = ctx.enter_context(tc.tile_pool(name="resp", bufs=4))
    wcp = ctx.enter_context(tc.tile_pool(name="wcp", bufs=12))
    psA = ctx.enter_context(tc.tile_pool(name="psA", bufs=2, space="PSUM"))
    psS = ctx.enter_context(tc.tile_pool(name="psS", bufs=2, space="PSUM"))

    BIG_SHAPE_BYTES = [P, 4 * NROWS * WPAD]  # cat_pad is the largest big tile

    # ---- constants / weights to SBUF ----
    idt = sing.tile([P, P], F32)
    nc.sync.dma_start(out=idt, in_=t["ident"][:])
    wq_sb, wk_sb, wv_sb, bq_sb, bk_sb = {}, {}, {}, {}, {}
    for b in (1, 2):
        wq_sb[b] = sing.tile([P, 2, 32], F32R, tag=f"wq{b}", name=f"wq{b}")
        nc.sync.dma_start(out=wq_sb[b], in_=t[f"wq{b}T"][:])
        wk_sb[b] = sing.tile([P, 2, 64], F32R, tag=f"wk{b}", name=f"wk{b}")
        nc.sync.dma_start(out=wk_sb[b], in_=t[f"wk{b}T"][:])
        wv_sb[b] = sing.tile([P, 2, C], F32R, tag=f"wv{b}", name=f"wv{b}")
        nc.sync.dma_start(out=wv_sb[b], in_=t[f"wv{b}T"][:])
        bq_sb[b] = sing.tile([32, 1], F32, tag=f"bq{b}", name=f"bq{b}")
        nc.sync.dma_start(out=bq_sb[b], in_=t[f"bq{b}"][:])
        bk_sb[b] = sing.tile([64, 1], F32, tag=f"bk{b}", name=f"bk{b}")
        nc.sync.dma_start(out=bk_sb[b], in_=t[f"bk{b}"][:])
    cinv_sb = sing.tile([P, 2], F32, tag="cinv")
    nc.sync.dma_start(out=cinv_sb, in_=t["cinv"][:])
    cbeta_sb = sing.tile([P, 2], F32, tag="cbeta")
    nc.sync.dma_start(out=cbeta_sb, in_=t["cbeta"][:])
    maskg_sb = sing.tile([P, 17], F32, tag="maskg")
    nc.sync.dma_start(out=maskg_sb, in_=t["maskg"][:])

    # ---- load x1, x2 (two column-half DMAs so the PE can start earlier) ----
    def load_x(name):
        x_sb = big.tile(BIG_SHAPE_BYTES, F32R, tag="big")
        xv = x_sb[:, : 2 * M].rearrange("p (kc n) -> p kc n", kc=2)
        src_ap = t[name][:].rearrange("(kc p) n -> p kc n", p=P)
        for c0 in range(0, M, 1024):
            nc.sync.dma_start(out=xv[:, :, ds(c0, 1024)],
                              in_=src_ap[:, :, ds(c0, 1024)])
        return xv

    x1_sb = load_x("x1")
    x2_sb = load_x("x2")

    # ---- k projections: k_b = wk_b @ x_b + bk_b, stored [128(c pad0), 4096] ----
    k_sb = {}
    for b, x_sb in ((1, x1_sb), (2, x2_sb)):
        kp = kqp.tile([P, M], F32R, tag=f"k{b}")
        for w0, ww in _wins(M, 512):
            ps = psS.tile([P, 1024], F32, tag="sc")
            for kc in range(2):
                _mm(nc, ps[0:64, :ww], wk_sb[b][:, kc, :],
                    x_sb[:, kc, ds(w0, ww)],
                    start=(kc == 0), stop=(kc == 1))
            nc.vector.tensor_scalar_add(kp[0:64, ds(w0, ww)], ps[0:64, :ww], bk_sb[b])
        k_sb[b] = kp

    # ---- vT projections: vT_b[m, c] = x_b.T @ wv_bT (no bias), plus ones col ----
    def make_vt(x_sb, b):
        vt = big.tile(BIG_SHAPE_BYTES, F32R, tag="big")
        vtv = vt[:, : 32 * 258].rearrange("p (mi c) -> p mi c", mi=32)
        nc.vector.memset(vtv[:, :, 256:257].bitcast(F32), 1.0)
        nc.vector.memset(vtv[:, :, 257:258].bitcast(F32), 0.0)
        for mi in range(32):
            ps_full = psS.tile([P, 1024], F32, tag="sc", name="vtps")
            ps = ps_full[:, :256]
            for kc in range(2):
                _mm(nc, ps, x_sb[:, kc, ts(mi, P)], wv_sb[b][:, kc, :],
                    start=(kc == 0), stop=(kc == 1))
            nc.vector.tensor_copy(out=vtv[:, mi, 0:256], in_=ps)
        return vtv

    # ---- q projection (shared by both branches): qp [128(c pad0), 2176] ----
    qp = kqp.tile([P, NQ], F32R, tag="qp")

    def q_half(name, b, row0):
        xq = big.tile(BIG_SHAPE_BYTES, F32R, tag="big")
        xqv = xq[:, : 2 * NQ].rearrange("p (kc n) -> p kc n", kc=2)
        xq_src = t[name][:].rearrange("(kc p) n -> p kc n", p=P)
        nc.sync.dma_start(out=xqv[:, :, 0:1088], in_=xq_src[:, :, 0:1088])
        nc.sync.dma_start(out=xqv[:, :, 1088:NQ], in_=xq_src[:, :, 1088:NQ])
        for w0, ww in QWINS:
            ps = psS.tile([P, 1024], F32, tag="sc")
            for kc in range(2):
                _mm(nc, ps[0:32, :ww], wq_sb[b][:, kc, :],
                    xqv[:, kc, ds(w0, ww)],
                    start=(kc == 0), stop=(kc == 1))
            nc.vector.tensor_scalar_add(qp[row0:row0 + 32, ds(w0, ww)],
                                        ps[0:32, :ww], bq_sb[b])

    q_half("x1q", 1, 0)
    vt1 = make_vt(x1_sb, 1)
    q_half("x2q", 2, 32)
    vt2 = make_vt(x2_sb, 2)

    # ---- cat_pad buffer [128, 4, 35*66], zeroed ----
    cat = big.tile(BIG_SHAPE_BYTES, F32R, tag="big")
    catv = cat[:].rearrange("p (i f) -> p i f", i=4)
    cat_r = cat[:].rearrange("p (i r w) -> p i r w", i=4, w=WPAD)
    nc.gpsimd.memset(cat[:].bitcast(F32), 0.0)

    # ---- attention branches ----
    for b, (kp, vtv, xr_name) in enumerate(
            [(k_sb[1], vt1, "x1r"), (k_sb[2], vt2, "x2r")]):
        for n0, nw in ATT_BLOCKS:
            nsub = nw // P
            g = 1024 // nw  # m-iters per exp group (4 for nw=256, 8 for 128)
            av = psA.tile([P, 1024], F32, tag="av")

            def flush_av(pend, av=av, vtv=vtv, nw=nw, nsub=nsub):
                g0, ex = pend
                for u in range(1024 // nw):
                    pmi = g0 + u
                    for j in range(nsub):
                        _mm(nc, av[:, ds(j * 512, 258)],
                            ex[:, ds(u * nw + j * P, P)], vtv[:, pmi, :],
                            start=(pmi == 0), stop=(pmi == 31))

            pend = None
            for g0 in range(0, 32, g):
                sc = psS.tile([P, 1024], F32, tag="sc")
                for u in range(g):
                    mi = g0 + u
                    _mm(nc, sc[:, ds(u * nw, nw)],
                        kp[0:64, ts(mi, P)], qp[0:64, ds(n0, nw)],
                        start=True, stop=True)
                ex = expp.tile([P, 1024], F32R, tag="ex")
                nc.scalar.activation(ex, sc, mybir.ActivationFunctionType.Exp)
                if pend is not None:
                    flush_av(pend)
                pend = (g0, ex)
            flush_av(pend)

            # epilogue per n-chunk of 128; transposes reuse the consumed AV bank
            for j in range(nsub):
                nch = n0 // P + j
                rs = scalp.tile([P, 1], F32, tag="rs")
                nc.vector.reciprocal(rs, av[:, ds(j * 512 + 256, 1)])
                nc.vector.tensor_mul(out=rs, in0=rs,
                                     in1=maskg_sb[:, ds(nch, 1)])
                nt = normp.tile([P, 256], F32, tag="nt")
                nc.vector.tensor_scalar_mul(nt, av[:, ds(j * 512, 256)], rs)
                rt = resp.tile([P, 2, P], F32, tag="rt")
                nc.sync.dma_start(
                    out=rt,
                    in_=t[xr_name][:].rearrange("(cc p) n -> p cc n", p=P)
                    [:, :, ts(nch, P)])
                for cc in range(2):
                    tp = av[:, ds(j * 512 + cc * P, P)]
                    nc.tensor.transpose(tp, nt[:, ts(cc, P)], idt)
                    nc.vector.tensor_add(
                        out=cat_r[:, 2 * b + cc, ds(2 * nch, 2), ds(1, 64)],
                        in0=tp.rearrange("p (r w) -> p r w", w=64),
                        in1=rt[:, cc, :].rearrange("p (r w) -> p r w", w=64))

        # write out this branch's attention output (rows 1..33 = the 32 real rows)
        ov = t[f"o{b + 1}"][:].rearrange("(cc p) h w -> p cc h w", p=P)
        for cc in range(2):
            nc.sync.dma_start(out=ov[:, cc],
                              in_=cat_r[:, 2 * b + cc, ds(1, 32), ds(1, 64)])

    # ---- conv 3x3 + BN + ReLU ----
    feat = big.tile(BIG_SHAPE_BYTES, F32, tag="big")
    featv = feat[:, : 2 * 2112].rearrange("p (o f) -> p o f", o=2)
    feat_r = feat[:, : 2 * 2112].rearrange("p (o r w) -> p o r w", o=2, w=WPAD)
    for oc in range(2):
        avc1 = psA.tile([P, 1024], F32, tag="av")
        avc2 = psA.tile([P, 1024], F32, tag="av")
        last = psS.tile([P, 1024], F32, tag="sc")

        def conv_dst(wi, ww, avc1=avc1, avc2=avc2, last=last):
            if wi < 2:
                return avc1[:, ds(wi * 512, ww)]
            if wi < 4:
                return avc2[:, ds((wi - 2) * 512, ww)]
            return last[:, :ww]

        wts = {}
        for ic in range(4):
            for tap in range(9):
                wt = wcp.tile([P, P], F32R, tag="wt", name=f"wt{oc}_{ic}_{tap}")
                nc.sync.dma_start(
                    out=wt, in_=t["wct"][tap // 3, tap % 3,
                                         ts(ic, P), ts(oc, P)])
                wts[(ic, tap)] = wt
        for ic in range(4):
            for tap in range(9):
                off = (tap // 3) * WPAD + (tap % 3) - 1
                for wi, (ws, ww) in enumerate(CONV_WINS):
                    _mm(nc, conv_dst(wi, ww), wts[(ic, tap)],
                        catv[:, ic, ds(ws + off, ww)],
                        start=(ic == 0 and tap == 0),
                        stop=(ic == 3 and tap == 8))
        for wi, (ws, ww) in enumerate(CONV_WINS):
            nc.scalar.activation(featv[:, oc, ds(ws, ww)], conv_dst(wi, ww),
                                 mybir.ActivationFunctionType.Relu,
                                 bias=cbeta_sb[:, ds(oc, 1)],
                                 scale=cinv_sb[:, ds(oc, 1)])
    fv = t["feat"][:].rearrange("(cc p) h w -> p cc h w", p=P)
    for oc in range(2):
        nc.sync.dma_start(out=fv[:, oc], in_=feat_r[:, oc, :, ds(1, 64)])


def _build():
    if "nc" in _CACHE:
        return _CACHE["nc"]
    nc = bacc.Bacc(None, target_bir_lowering=False)
    t = _declare_io(nc)
    from contextlib import ExitStack
    with tile.TileContext(nc) as tc, ExitStack() as ctx:
        _emit(nc, tc, t, ctx)
    nc.finalize()
    _CACHE["nc"] = nc
    return nc


def _prep_host(inputs):
    d = {k: np.ascontiguousarray(np.asarray(v, np.float32)) for k, v in inputs.items()}
    gamma = float(d["gamma"].reshape(-1)[0])
    inv = d["bn_scale"] / np.sqrt(d["bn_var"] + EPS)
    beta = d["bn_bias"] - d["bn_mean"] * inv

    def chunked(w):  # [256, o] -> [128, 2, o]
        return np.ascontiguousarray(w.reshape(2, P, -1).transpose(1, 0, 2))

    shared = {
        "wq1T": chunked(d["wq1"].T), "wq2T": chunked(d["wq2"].T),
        "wk1T": chunked(d["wk1"].T), "wk2T": chunked(d["wk2"].T),
        "wv1T": chunked(d["wv1"].T), "wv2T": chunked(d["wv2"].T),
        "bq1": d["bq1"].reshape(32, 1).copy(), "bq2": d["bq2"].reshape(32, 1).copy(),
        "bk1": d["bk1"].reshape(64, 1).copy(), "bk2": d["bk2"].reshape(64, 1).copy(),
        "cinv": np.ascontiguousarray(inv.reshape(2, P).T),
        "cbeta": np.ascontiguousarray(beta.reshape(2, P).T),
        "wct": np.ascontiguousarray(d["w_cat"].transpose(2, 3, 1, 0)),
        "ident": np.eye(P, dtype=np.float32),
    }
    gbv = {1: gamma * d["bv1"], 2: gamma * d["bv2"]}

    in_maps = []
    for core in range(8):
        s, half = core // 2, core % 2
        h0 = 32 * half
        x1 = np.ascontiguousarray(d["input1"][s].reshape(C, M))
        x2 = np.ascontiguousarray(d["input2"][s].reshape(C, M))
        n_lo, n_hi = (h0 - 1) * 64, (h0 + 33) * 64
        lo_pad, hi_pad = max(0, -n_lo), max(0, n_hi - M)
        sl = slice(n_lo + lo_pad, n_hi - hi_pad)

        def pad_slice(x, add=None):
            o = np.zeros((C, NQ), np.float32)
            body = x[:, sl]
            if add is not None:
                body = body + add[:, None]
            o[:, lo_pad:NQ - hi_pad] = body
            return o

        maskg = np.zeros(NQ, np.float32)
        maskg[lo_pad:NQ - hi_pad] = gamma
        m = dict(shared)
        m.update({
            "x1": x1, "x2": x2,
            "x1q": pad_slice(x1), "x2q": pad_slice(x2),
            "x1r": pad_slice(x1, gbv[1]), "x2r": pad_slice(x2, gbv[2]),
            "maskg": np.ascontiguousarray(maskg.reshape(17, P).T),
        })
        in_maps.append(m)
    return in_maps


def _run_cached_pjrt(nc, in_maps):
    """run_bass_via_pjrt equivalent with the traced/jitted executable cached
    across kernel() calls (run_bass_via_pjrt rebuilds it every call)."""
    import jax
    import numpy as _np
    from jax.sharding import Mesh, PartitionSpec
    from jax.experimental.shard_map import shard_map
    from concourse import bass2jax, mybir as _mb

    n_cores = len(in_maps)
    if "pjrt" not in _CACHE:
        bass2jax.install_neuronx_cc_hook()
        in_names, out_names, out_avals, zero_shapes = [], [], [], []
        for alloc in nc.m.functions[0].allocations:
            if not isinstance(alloc, _mb.MemoryLocationSet):
                continue
            name = alloc.memorylocations[0].name
            if alloc.kind == "ExternalInput":
                if nc.partition_id_tensor is None or \
                        name != nc.partition_id_tensor.name:
                    in_names.append(name)
            elif alloc.kind == "ExternalOutput":
                out_names.append(name)
                shape = tuple(alloc.tensor_shape)
                dtype = _mb.dt.np(alloc.dtype)
                out_avals.append(jax.core.ShapedArray(shape, dtype))
                zero_shapes.append((shape, dtype))
        n_params = len(in_names)
        all_names = in_names + out_names
        pid_name = nc.partition_id_tensor.name if nc.partition_id_tensor else None
        if pid_name is not None:
            all_names = all_names + [pid_name]

        def _body(*args):
            operands = list(args)
            if pid_name is not None:
                operands.append(bass2jax.partition_id_tensor())
            outs = bass2jax._bass_exec_p.bind(
                *operands,
                out_avals=tuple(out_avals),
                in_names=tuple(all_names),
                out_names=tuple(out_names),
                lowering_input_output_aliases=(),
                sim_require_finite=True,
                sim_require_nnan=True,
                nc=nc,
            )
            return tuple(outs)

        devices = jax.devices()[:n_cores]
        mesh = Mesh(_np.asarray(devices), ("core",))
        n_outs = len(out_names)
        sharded = jax.jit(
            shard_map(_body, mesh=mesh,
                      in_specs=(PartitionSpec("core"),) * (n_params + n_outs),
                      out_specs=(PartitionSpec("core"),) * n_outs,
                      check_rep=False),
            donate_argnums=tuple(range(n_params, n_params + n_outs)),
            keep_unused=True,
        )
        _CACHE["pjrt"] = (sharded, in_names, out_names, out_avals, zero_shapes)

    sharded, in_names, out_names, out_avals, zero_shapes = _CACHE["pjrt"]
    n_cores_ax = len(in_maps)
    concat_in = [
        _np.concatenate([_np.asarray(in_maps[c][nm]) for c in range(n_cores_ax)], axis=0)
        for nm in in_names
    ]
    concat_zeros = [
        _np.zeros((n_cores_ax * s[0], *s[1:]), d) for s, d in zero_shapes
    ]
    out_arrs = sharded(*concat_in, *concat_zeros)
    return [
        {nm: _np.asarray(out_arrs[i]).reshape(n_cores_ax, *out_avals[i].shape)[c]
         for i, nm in enumerate(out_names)}
        for c in range(n_cores_ax)
    ]


def kernel(**inputs):
    nc = _build()
    in_maps = _prep_host(inputs)
    try:
        results = _run_cached_pjrt(nc, in_maps)
    except Exception:
        _CACHE.pop("pjrt", None)
        res = run_bass_kernel_spmd(nc, in_maps, core_ids=list(range(8)))
        _CACHE["last_results"] = res
        results = res.results
    feat = np.zeros((4, C, 64, 64), np.float32)
    o1 = np.zeros((4, C, 64, 64), np.float32)
    o2 = np.zeros((4, C, 64, 64), np.float32)
    for core in range(8):
        s, half = core // 2, core % 2
        r = results[core]
        feat[s, :, 32 * half:32 * half + 32] = r["feat"]
        o1[s, :, 32 * half:32 * half + 32] = r["o1"]
        o2[s, :, 32 * half:32 * half + 32] = r["o2"]
    return (feat, o1, o2)
